# revision 1
# baseline (speedup 1.0000x reference)
"""Trainium2 kernel for nn_Contrast: contrastive loss over a 10000x10000
exp-cosine-similarity matrix, sharded by rows across 8 NeuronCores.

Structure:
  host (tiny, O(N*D)): 8->8->8 MLP projection of both views, row norms,
      fold 1/(n1*n2*tau) into the operands:  a = zp1/n1,  b = zp2/(n2*tau).
      Then m[i,j] = exp(a_i . b_j).
  device (O(N^2)), per core k over its 1280-row slice of a:
      for each [128 x <=512] tile of a_rows @ b^T:
        PE matmul (K=8) -> PSUM (3 tiles share a 3-bank PSUM tensor)
        ACT exp PSUM->SBUF with accum_out => row-sum partials (free)
        PE one-hot matmul (E_c^T @ exp_tile) accumulating column sums for
        all tiles into a single [20, 512] PSUM bank
  host: subtract zero-padding contributions, add eps, diag from exact dots,
      assemble the two mean log-ratio losses.
"""

import numpy as np

import concourse.bass as bass
import concourse.bacc as bacc
import concourse.mybir as mybir
import concourse.tile as tile
from concourse.bass_utils import run_bass_kernel_spmd

TAU = 0.5
LAM = 0.5
EPS = 1e-8

N = 10000
D = 8
NCORES = 8
RPAD = 10240              # lhs rows padded: 8 cores * 1280
RPC = RPAD // NCORES      # rows per core = 1280
NSTRIP = RPC // 128       # 10 strips of 128 rows
ROW_PAD = RPAD - N        # 240 zero lhs rows -> contribute exp(0)=1 per column

# column tiles cover exactly N columns: 19 x 512 + 272
COL_TILES = [(c * 512, min(512, N - c * 512)) for c in range((N + 511) // 512)]
NCT = len(COL_TILES)      # 20
# tiles per strip are grouped so each group's matmuls share one PSUM tensor
# and one ACT(exp) instruction. PSUM budget is 8 banks: 2 x 3-bank tensors
# (double-buffered) + 1 bank for the column-sum accumulator. The short
# (2-tile, 784-wide) group goes first in each strip: a short ACT instruction
# in the middle of a strip stalls the pipeline less there.
GROUPS = [COL_TILES[18:20]] + [COL_TILES[i : i + 3] for i in range(0, 18, 3)]

# "f32" is the exact-but-slow path (PE runs fp32 at 4 cycles/row).
# "f32r" streams fp32 bits through the PE at full rate with relaxed rounding;
# "bf16" is the same speed with coarser rounding and no staging copies.
# All accumulation stays fp32 and the scalar loss averages the per-element
# rounding noise away (measured loss rel err: bf16 0.0, f32r 1e-7, f32 1e-7).
MM_DTYPE = "bf16"


def _mybir_dt(name):
    return {
        "f32": mybir.dt.float32,
        "f32r": mybir.dt.float32r,
        "bf16": mybir.dt.bfloat16,
    }[name]


def _np_dt(name):
    if name in ("f32", "f32r"):
        return np.float32
    import ml_dtypes

    return ml_dtypes.bfloat16


def _build_nc(dt_name):
    dt_in = _mybir_dt(dt_name)
    f32 = mybir.dt.float32
    nc = bacc.Bacc(None)

    dram_dt = mybir.dt.bfloat16 if dt_name == "bf16" else mybir.dt.float32
    lhsT = nc.dram_tensor("lhsT", [D, RPC], dram_dt, kind="ExternalInput")
    rhsT = nc.dram_tensor("rhsT", [D, N], dram_dt, kind="ExternalInput")
    eblk = nc.dram_tensor("eblk", [128, NCT * 20], dram_dt, kind="ExternalInput")
    out_rowsum = nc.dram_tensor("out_rowsum", [128, NSTRIP], f32, kind="ExternalOutput")
    out_colsum = nc.dram_tensor("out_colsum", [20, 512], f32, kind="ExternalOutput")

    ngroups = len(GROUPS)
    n_onehot = NSTRIP * NCT

    with tile.TileContext(nc) as tc:
        with (
            tc.tile_pool(name="inp", bufs=1) as inp_pool,
            tc.tile_pool(name="etile", bufs=4) as etile_pool,
            tc.tile_pool(name="rowp", bufs=2) as rowp_pool,
            tc.tile_pool(name="persist", bufs=1) as persist_pool,
            tc.tile_pool(name="pmm", bufs=2, space="PSUM") as pmm_pool,
            tc.tile_pool(name="pcol", bufs=1, space="PSUM") as pcol_pool,
        ):
            lhsT_sb = inp_pool.tile([D, RPC], dt_in)
            rhsT_sb = inp_pool.tile([D, N], dt_in)
            eblk_sb = inp_pool.tile([128, NCT * 20], dt_in)

            if dt_name == "f32r":
                # f32r operands need a rounding producer; sync-DMA into f32
                # staging, then idle-DVE copies do the cast. Chunked so the
                # first matmuls start as soon as their span is staged; eblk is
                # only needed by the first one-hot matmul (~8us in), so it
                # loads after the first two rhs chunks.
                lhsT_st = inp_pool.tile([D, RPC], f32)
                rhsT_st = inp_pool.tile([D, N], f32)
                eblk_st = inp_pool.tile([128, NCT * 20], f32)

                # each dma_start costs ~650ns of serial sequencer issue, so
                # the pieces feeding the first matmuls go first and the bulk
                # follows in a few large DMAs. DVE cast copies are chunked in
                # group-consumption order so compute starts as data rounds.
                spans = []
                for grp in GROUPS:
                    g0 = grp[0][0]
                    spans.append((g0, g0 + sum(w for _, w in grp)))
                rest = sorted(spans[2:])  # contiguous ascending tail spans
                nc.sync.dma_start(out=lhsT_st[:, 0:128], in_=lhsT[:, 0:128])
                nc.sync.dma_start(
                    out=rhsT_st[:, spans[0][0] : spans[0][1]],
                    in_=rhsT[:, spans[0][0] : spans[0][1]],
                )
                nc.sync.dma_start(
                    out=rhsT_st[:, spans[1][0] : spans[1][1]],
                    in_=rhsT[:, spans[1][0] : spans[1][1]],
                )
                nc.sync.dma_start(out=lhsT_st[:, 128:RPC], in_=lhsT[:, 128:RPC])
                nc.sync.dma_start(
                    out=rhsT_st[:, rest[0][0] : rest[2][1]],
                    in_=rhsT[:, rest[0][0] : rest[2][1]],
                )
                nc.sync.dma_start(out=eblk_st[:], in_=eblk[:])
                nc.sync.dma_start(
                    out=rhsT_st[:, rest[3][0] : rest[-1][1]],
                    in_=rhsT[:, rest[3][0] : rest[-1][1]],
                )

                def _cast(dst, st, lo, hi):
                    nc.vector.tensor_copy(out=dst[:, lo:hi], in_=st[:, lo:hi])

                _cast(lhsT_sb, lhsT_st, 0, 128)
                _cast(rhsT_sb, rhsT_st, *spans[0])
                _cast(lhsT_sb, lhsT_st, 128, RPC)
                _cast(rhsT_sb, rhsT_st, *spans[1])
                _cast(eblk_sb, eblk_st, 0, NCT * 20)
                for sp in spans[2:]:
                    _cast(rhsT_sb, rhsT_st, *sp)
            else:
                nc.sync.dma_start(out=lhsT_sb[:], in_=lhsT[:])
                for grp in GROUPS[:2]:
                    g0 = grp[0][0]
                    gw = sum(w for _, w in grp)
                    nc.sync.dma_start(
                        out=rhsT_sb[:, g0 : g0 + gw], in_=rhsT[:, g0 : g0 + gw]
                    )
                nc.sync.dma_start(out=eblk_sb[:], in_=eblk[:])
                for grp in GROUPS[2:]:
                    g0 = grp[0][0]
                    gw = sum(w for _, w in grp)
                    nc.sync.dma_start(
                        out=rhsT_sb[:, g0 : g0 + gw], in_=rhsT[:, g0 : g0 + gw]
                    )

            rowsum_sb = persist_pool.tile([128, NSTRIP], f32)
            colsum_sb = persist_pool.tile([20, 512], f32)
            colp = pcol_pool.tile([20, 512], f32)

            # software-pipeline the one-hot (column-sum) matmuls two groups
            # behind the main matmuls: at strip boundaries PE then runs the
            # next strip's main matmuls before the deferred one-hots, so ACT
            # is never left waiting on PE's in-order queue
            pending = []
            onehot_idx = 0

            def flush_one(et, grp):
                nonlocal onehot_idx
                off = 0
                for c0, w in grp:
                    c = c0 // 512  # global column-tile index = colp row
                    nc.tensor.matmul(
                        colp[:, 0:w],
                        eblk_sb[:, c * 20 : (c + 1) * 20],
                        et[:, off : off + w],
                        start=(onehot_idx == 0),
                        stop=(onehot_idx == n_onehot - 1),
                        skip_group_check=True,
                    )
                    off += w
                    onehot_idx += 1

            def flush_pending(keep=0):
                while len(pending) > keep:
                    flush_one(*pending.pop(0))

            for r in range(NSTRIP):
                rowp = rowp_pool.tile([128, ngroups], f32)
                for gi, grp in enumerate(GROUPS):
                    gw = sum(w for _, w in grp)
                    pa = pmm_pool.tile([128, 1536], f32, name=f"pa_{r}_{gi}", tag="pa")
                    off = 0
                    for c0, w in grp:
                        nc.tensor.matmul(
                            pa[:, off : off + w],
                            lhsT_sb[:, r * 128 : (r + 1) * 128],
                            rhsT_sb[:, c0 : c0 + w],
                            start=True,
                            stop=True,
                        )
                        off += w
                    et = etile_pool.tile([128, 1536], dt_in)
                    nc.scalar.activation(
                        et[:, :gw],
                        pa[:, :gw],
                        mybir.ActivationFunctionType.Exp,
                        accum_out=rowp[:, gi : gi + 1],
                    )
                    flush_pending(keep=1)
                    pending.append((et, grp))
                nc.vector.reduce_sum(
                    out=rowsum_sb[:, r : r + 1],
                    in_=rowp[:, :],
                    axis=mybir.AxisListType.X,
                )
            flush_pending()

            nc.vector.tensor_copy(out=colsum_sb[:], in_=colp[:])
            nc.sync.dma_start(out=out_rowsum[:], in_=rowsum_sb[:])
            nc.sync.dma_start(out=out_colsum[:], in_=colsum_sb[:])

    nc.compile()
    return nc


_NC_CACHE = {}


def _get_nc(dt_name):
    if dt_name not in _NC_CACHE:
        _NC_CACHE[dt_name] = _build_nc(dt_name)
    return _NC_CACHE[dt_name]


def _proj_np(z, W1, b1, W2, b2):
    h = z @ W1.T + b1
    h = np.where(h > 0, h, np.expm1(h)).astype(np.float32)
    return (h @ W2.T + b2).astype(np.float32)


def _prepare_operands(z_mp, z_sc, W1, b1, W2, b2):
    zp1 = _proj_np(z_mp.astype(np.float32), W1, b1, W2, b2)
    zp2 = _proj_np(z_sc.astype(np.float32), W1, b1, W2, b2)
    n1 = np.sqrt(np.sum(zp1 * zp1, axis=1, keepdims=True)).astype(np.float32)
    n2 = np.sqrt(np.sum(zp2 * zp2, axis=1, keepdims=True)).astype(np.float32)
    a = (zp1 / n1).astype(np.float32)
    b = (zp2 / (n2 * np.float32(TAU))).astype(np.float32)
    dots = np.sum(a * b, axis=1).astype(np.float32)  # diag logits (exact path)
    return a, b, dots


def _make_in_maps(a, b):
    np_dt = _np_dt(MM_DTYPE)
    a_pad = np.zeros((RPAD, D), np.float32)
    a_pad[:N] = a
    aT = np.ascontiguousarray(a_pad.T).astype(np_dt)
    bT = np.ascontiguousarray(b.T).astype(np_dt)
    E = np.ascontiguousarray(
        np.tile(np.eye(20, dtype=np_dt)[None], (128, 1, 1)).reshape(128, NCT * 20)
    )
    return [
        {
            "lhsT": np.ascontiguousarray(aT[:, k * RPC : (k + 1) * RPC]),
            "rhsT": bT,
            "eblk": E,
        }
        for k in range(NCORES)
    ]


def _finalize(res, dots):
    rowsum_full = np.concatenate(
        [np.asarray(res[k]["out_rowsum"]).T.reshape(-1) for k in range(NCORES)]
    )
    colsum_full = np.sum(
        [np.asarray(res[k]["out_colsum"]).reshape(-1) for k in range(NCORES)], axis=0
    )
    row_sum = rowsum_full[:N].astype(np.float64) + EPS
    col_sum = colsum_full[:N].astype(np.float64) - ROW_PAD + EPS
    diag = np.exp(dots.astype(np.float64))
    lori_mp = -np.mean(np.log(diag / row_sum))
    lori_sc = -np.mean(np.log(diag / col_sum))
    return np.float32(LAM * lori_mp + (1.0 - LAM) * lori_sc)


def kernel(z_mp, z_sc, W1, b1, W2, b2):
    a, b, dots = _prepare_operands(z_mp, z_sc, W1, b1, W2, b2)
    in_maps = _make_in_maps(a, b)
    nc = _get_nc(MM_DTYPE)
    res = run_bass_kernel_spmd(nc, in_maps, list(range(NCORES))).results
    return _finalize(res, dots)



# revision 3
# speedup vs baseline: 1.0534x; 1.0534x over previous
"""Trainium2 kernel for nn_Contrast: contrastive loss over a 10000x10000
exp-cosine-similarity matrix, sharded by rows across 8 NeuronCores.

Device pipeline per core (1280-row slice, 10 strips of 128 rows):
  PE:  fp8e4m3 DoubleRow matmuls (K=8 split as 2 k-tiles of 4) compute
       logits*32 into PSUM at 0.5 cycles/col; paired one-hot DoubleRow
       matmuls (two 512-col tiles as the two k-tiles) reduce each strip's
       fp8 exp tile over rows into a [20,512] colsum PSUM tile.
  ACT: exp(psum/32) for strip cols [0:A_END) -> fp8 et, accum_out gives
       f32 row-sum partials for free.
  DVE: Schraudolph exp for cols [A_END:10000): one tensor_scalar computes
       round(psum*A + B) as int8, whose bits ARE the fp8 exp approximation
       (rel err ~4%, averages out in the 10k-col sums); a second in-place
       tensor_scalar with accum_out row-sums those fp8 values at the
       2-elem/cycle all-SBUF rate; also drains colsum PSUM into SBUF f32.
Host: 8->8->8 MLP projection of both views (tiny), fp8 operand prep, exact
  diag dots, gather + pad correction + log/mean finalize.
"""

import numpy as np

import concourse.bass as bass
import concourse.bacc as bacc
import concourse.mybir as mybir
import concourse.tile as tile
from concourse.bass_utils import run_bass_kernel_spmd

TAU = 0.5
LAM = 0.5
EPS = 1e-8

N = 10000
D = 8
NCORES = 8
RPC = 1280                 # rows per core (8*1280 = 10240, 240 zero pad rows)
NSTRIP = RPC // 128        # 10
ROW_PAD = NCORES * RPC - N  # 240 zero lhs rows in core 7

SA, SB = 8.0, 4.0          # fp8 operand scales; psum = 32 * logit
PSCALE = SA * SB
A_S = float(D / (PSCALE * np.log(2.0)))  # schraudolph slope (8/(32 ln2))
B2 = 55.5625               # schraudolph offset (fp8e4m3 bias, calibrated)

A_END = 6272               # strip cols [0:A_END) on ACT, rest on DVE
NT = 20                    # 512-col tiles per strip row (last = 272)
TILE_W = [512] * 19 + [272]
TILE_OFF = np.cumsum([0] + [2 * w for w in TILE_W]).tolist()  # rhs layout offs
RHS_LEN = TILE_OFF[-1]     # 20000
# groups of 4 tiles -> one [128, <=2048] psum tensor
GROUPS = [list(range(4 * g, 4 * g + 4)) for g in range(5)]
GCOL = [sum(TILE_W[t] for ts_ in GROUPS[:g] for t in ts_) for g in range(6)]
# pairs for one-hot colsum: (2p, 2p+1); pair 9 halves are 392 wide
PAIR_W = [512] * 9 + [392]

f32 = mybir.dt.float32
fp8 = mybir.dt.float8e4
i8 = mybir.dt.int8
u8 = mybir.dt.uint8


def _build_nc():
    nc = bacc.Bacc(None)
    lhsT = nc.dram_tensor("lhsT", [4, 2 * 128 * NSTRIP], fp8, kind="ExternalInput")
    rhsT = nc.dram_tensor("rhsT", [4, RHS_LEN], fp8, kind="ExternalInput")
    eblk = nc.dram_tensor("eblk", [128, 64 * 10], fp8, kind="ExternalInput")
    out_rows = nc.dram_tensor("out_rows", [128, 5 * NSTRIP], f32, kind="ExternalOutput")
    out_colsum = nc.dram_tensor("out_colsum", [20, 512], f32, kind="ExternalOutput")

    with tile.TileContext(nc) as tc:
        with (
            tc.tile_pool(name="inp", bufs=1) as inp_pool,
            tc.tile_pool(name="etp", bufs=2) as et_pool,
            tc.tile_pool(name="persist", bufs=1) as persist_pool,
            tc.tile_pool(name="pmm", bufs=2, space="PSUM") as pmm_pool,
        ):
            lhsT_sb = inp_pool.tile([4, 2 * 128 * NSTRIP], fp8)
            rhsT_sb = inp_pool.tile([4, RHS_LEN], fp8)
            eblk_sb = inp_pool.tile([128, 64 * 10], fp8)

            nc.sync.dma_start(out=lhsT_sb[:], in_=lhsT[:])
            for lo, hi in [(0, 4096), (4096, 8192), (8192, 12288),
                           (12288, 16384), (16384, RHS_LEN)]:
                nc.sync.dma_start(out=rhsT_sb[:, lo:hi], in_=rhsT[:, lo:hi])
            nc.sync.dma_start(out=eblk_sb[:], in_=eblk[:])

            rows_sb = persist_pool.tile([128, 5 * NSTRIP], f32)
            colsum_sb = persist_pool.tile([20, 512], f32)
            nc.vector.memset(colsum_sb[:], 0.0)

            pending_oh = []  # et tiles awaiting one-hot colsum reduction

            def emit_onehots(et):
                colp = pmm_pool.tile([32, 512], f32, name="colp", tag="pa")
                for p in range(10):
                    w = PAIR_W[p]
                    nc.tensor.matmul(
                        colp[:, 0:w],
                        eblk_sb[:, 64 * p : 64 * p + 64].rearrange(
                            "k (t m) -> k t m", t=2
                        ),
                        et[:, 1024 * p : 1024 * p + 2 * w].rearrange(
                            "q (t n) -> q t n", t=2
                        ),
                        start=True, stop=True,
                        perf_mode=mybir.MatmulPerfMode.DoubleRow,
                        skip_group_check=True,
                    )
                # rows 18-19 cols [392:512) accumulate stale psum (finite);
                # the host reads only the valid cells
                nc.vector.tensor_tensor(
                    out=colsum_sb[0:20, :], in0=colsum_sb[0:20, :],
                    in1=colp[0:20, 0:512], op=mybir.AluOpType.add,
                )

            for r in range(NSTRIP):
                lh = lhsT_sb[:, 256 * r : 256 * r + 256].rearrange(
                    "k (t m) -> k t m", t=2
                )
                et = et_pool.tile([128, GCOL[5]], fp8, name=f"et{r % 2}", tag="et")
                pas = []
                for g in range(5):
                    gw = GCOL[g + 1] - GCOL[g]
                    pa = pmm_pool.tile([128, 2048], f32, name=f"pa_{r}_{g}", tag="pa")
                    off = 0
                    for t in GROUPS[g]:
                        w = TILE_W[t]
                        nc.tensor.matmul(
                            pa[:, off : off + w],
                            lh,
                            rhsT_sb[:, TILE_OFF[t] : TILE_OFF[t] + 2 * w].rearrange(
                                "k (t2 n) -> k t2 n", t2=2
                            ),
                            start=True, stop=True,
                            perf_mode=mybir.MatmulPerfMode.DoubleRow,
                            skip_group_check=True,
                        )
                        off += w
                    pas.append(pa)
                    if g < 3:
                        # full ACT group: exp + f32 rowsum accumulation
                        nc.scalar.activation(
                            et[:, GCOL[g] : GCOL[g] + gw], pa[:, :gw],
                            mybir.ActivationFunctionType.Exp,
                            scale=1.0 / PSCALE,
                            accum_out=rows_sb[:, 4 * r + g : 4 * r + g + 1],
                        )
                    if g == 2 and pending_oh:
                        emit_onehots(pending_oh.pop())
                # g3: ACT takes [6144:A_END), DVE the rest
                acut = A_END - GCOL[3]
                nc.scalar.activation(
                    et[:, GCOL[3] : A_END], pas[3][:, 0:acut],
                    mybir.ActivationFunctionType.Exp,
                    scale=1.0 / PSCALE,
                    accum_out=rows_sb[:, 4 * r + 3 : 4 * r + 4],
                )
                nc.vector.tensor_scalar(
                    out=et[:, A_END : GCOL[4]].bitcast(i8),
                    in0=pas[3][:, acut:2048],
                    scalar1=A_S, scalar2=B2,
                    op0=mybir.AluOpType.mult, op1=mybir.AluOpType.add,
                )
                nc.vector.tensor_scalar(
                    out=et[:, GCOL[4] : GCOL[5]].bitcast(i8),
                    in0=pas[4][:, 0 : GCOL[5] - GCOL[4]],
                    scalar1=A_S, scalar2=B2,
                    op0=mybir.AluOpType.mult, op1=mybir.AluOpType.add,
                )
                # fp8 rowsum of the schraudolph span (2x all-SBUF path)
                nc.vector.tensor_scalar(
                    out=et[:, A_END : GCOL[5]], in0=et[:, A_END : GCOL[5]],
                    scalar1=1.0, scalar2=0.0,
                    op0=mybir.AluOpType.mult, op1=mybir.AluOpType.add,
                    accum_out=rows_sb[:, 40 + r : 41 + r],
                )
                pending_oh.append(et)

            emit_onehots(pending_oh.pop())
            nc.sync.dma_start(out=out_rows[:], in_=rows_sb[:])
            nc.sync.dma_start(out=out_colsum[:], in_=colsum_sb[:])

    nc.compile()
    return nc


_NC_CACHE = {}
MM_DTYPE = "fp8dr"


def _get_nc(dt_name=MM_DTYPE):
    if dt_name not in _NC_CACHE:
        _NC_CACHE[dt_name] = _build_nc()
    return _NC_CACHE[dt_name]


def _proj_np(z, W1, b1, W2, b2):
    h = z @ W1.T + b1
    h = np.where(h > 0, h, np.expm1(h)).astype(np.float32)
    return (h @ W2.T + b2).astype(np.float32)


def _prepare_operands(z_mp, z_sc, W1, b1, W2, b2):
    zp1 = _proj_np(z_mp.astype(np.float32), W1, b1, W2, b2)
    zp2 = _proj_np(z_sc.astype(np.float32), W1, b1, W2, b2)
    n1 = np.sqrt(np.sum(zp1 * zp1, axis=1, keepdims=True)).astype(np.float32)
    n2 = np.sqrt(np.sum(zp2 * zp2, axis=1, keepdims=True)).astype(np.float32)
    a = (zp1 / n1).astype(np.float32)
    b = (zp2 / (n2 * np.float32(TAU))).astype(np.float32)
    dots = np.sum(a.astype(np.float64) * b, axis=1)  # exact diag logits
    return a, b, dots


def _np_fp8():
    return mybir.dt.np(fp8)


def _make_in_maps(a, b):
    np8 = _np_fp8()
    a_pad = np.zeros((NCORES * RPC, D), np.float32)
    a_pad[:N] = a * SA
    a8 = a_pad.astype(np8)
    b8 = (b * SB).astype(np8)

    # lhsT per core: [4, (strip, t, m)] with element [k, r, t, m] = a8[r*128+m, 4t+k]
    lhs_all = (
        a8.reshape(NCORES, NSTRIP, 128, 2, 4)   # [core, r, m, t, k]
        .transpose(0, 4, 1, 3, 2)               # [core, k, r, t, m]
        .reshape(NCORES, 4, 2 * 128 * NSTRIP)
    )
    # rhsT blocked per 512-tile: [4, (tile, t, n)]
    parts = []
    for t in range(NT):
        w = TILE_W[t]
        sub = b8[512 * t : 512 * t + w]          # [w, 8]
        parts.append(sub.reshape(w, 2, 4).transpose(2, 1, 0).reshape(4, 2 * w))
    rhsT = np.ascontiguousarray(np.concatenate(parts, axis=1))

    E = np.zeros((128, 10, 2, 32), np8)
    for p in range(10):
        E[:, p, 0, 2 * p] = 1.0
        E[:, p, 1, 2 * p + 1] = 1.0
    eblk = np.ascontiguousarray(E.reshape(128, 640))

    return [
        {"lhsT": np.ascontiguousarray(lhs_all[k]), "rhsT": rhsT, "eblk": eblk}
        for k in range(NCORES)
    ]


def _finalize(res, dots):
    # row sums: [core][strip, partition] = sum of 4 ACT partials + DVE partial
    rows = []
    for k in range(NCORES):
        m = np.asarray(res[k]["out_rows"]).astype(np.float64)  # [128, 50]
        acts = m[:, :40].reshape(128, NSTRIP, 4).sum(axis=2)   # [128, r]
        dves = m[:, 40:50]                                     # [128, r]
        rows.append((acts + dves).T.reshape(-1))               # row-major r*128+p
    row_sum = np.concatenate(rows)[:N] + EPS

    # col sums: sum cores, then unmap pair layout
    cs = np.sum([np.asarray(res[k]["out_colsum"]) for k in range(NCORES)], axis=0)
    cs = cs.astype(np.float64)  # [20, 512]
    col_sum = np.empty(N, np.float64)
    for p in range(9):
        col_sum[1024 * p : 1024 * p + 512] = cs[2 * p, :]
        col_sum[1024 * p + 512 : 1024 * p + 1024] = cs[2 * p + 1, :]
    col_sum[9216:9608] = cs[18, :392]
    col_sum[9608:10000] = cs[19, :392]
    # zero pad rows contribute exp(0)=1 in ACT cols and fp8(round(B2))=1.0 in
    # DVE cols (B2 rounds to 56 -> bits of 1.0)
    col_sum = col_sum - ROW_PAD + EPS

    diag = np.exp(dots)
    lori_mp = -np.mean(np.log(diag / row_sum))
    lori_sc = -np.mean(np.log(diag / col_sum))
    return np.float32(LAM * lori_mp + (1.0 - LAM) * lori_sc)


def kernel(z_mp, z_sc, W1, b1, W2, b2):
    a, b, dots = _prepare_operands(z_mp, z_sc, W1, b1, W2, b2)
    in_maps = _make_in_maps(a, b)
    nc = _get_nc()
    res = run_bass_kernel_spmd(nc, in_maps, list(range(NCORES))).results
    return _finalize(res, dots)
